# revision 20
# baseline (speedup 1.0000x reference)
"""DeepAR 2-layer LSTM (B=16, L_IN=96, L_OUT=24, N=320, H=128) on 8 TRN2 cores.

Strategy (data-parallel over B*N=5120 rows, 640 rows/core):
  - Layout: hidden/gate index on SBUF partitions, batch rows on the free dim.
    h, c are [128, 640] tiles; gates are computed as W.T-slices (lhsT) against
    h (rhs) so no transposes are ever needed.
  - The input-side transform of layer 0 is folded through the embedding:
    x_eff[t] = [tgt_t, cov_{t+1}] (5 values/row) with W0eff = [Wih0[:, :E] @ We,
    Wih0[:, E:]] -> K=5 matmuls, packed 4-way with tile_position row strips.
  - Gate order is permuted to (i, f, o, g) host-side.
  - bf16 matmul operands (1 cyc/row on PE); cell state c and all gate
    nonlinearities in fp32.
  - Engines: PE matmuls; ACT sigmoid/tanh (bottleneck ~7us/step); DVE does
    p=si*tg, c=p+q, h=so*tc; GPSIMD does q=sf*c_prev.
  - Head (last 24 steps) runs post-scan from h1 history kept in SBUF; softplus
    on device; bias folded in via a K=1 matmul against a ones row.
"""

import os

import numpy as np

B, L_IN, L_OUT, N_SER, COV = 16, 96, 24, 320, 4
E, H = 64, 128
T_STEPS = 119  # L_IN + L_OUT - 1
NCORES = 8
BN = B * N_SER          # 5120
R = BN // NCORES        # 640 rows per core
G4 = 4 * H              # 512 gates

_PROGRAM_CACHE: dict = {}


def _gate_perm() -> np.ndarray:
    # torch gate order in weights: i, f, g, o -> reorder rows to (f, g, i, o)
    # so the critical-path gates (forget, cell) come out of the PE first
    idx = np.arange(G4).reshape(4, H)
    return np.concatenate([idx[1], idx[2], idx[0], idx[3]])


def _build_program(t_steps: int, scan_steps: int | None = None, repeat: int = 1):
    import concourse.bacc as bacc
    import concourse.mybir as mybir
    import concourse.tile as tile

    f32 = mybir.dt.float32
    bf16 = mybir.dt.bfloat16
    AF = mybir.ActivationFunctionType

    nc = bacc.Bacc()

    xrep_d = nc.declare_dram_parameter("xrep", [t_steps, 20, R], bf16, isOutput=False)
    whh0t_d = nc.declare_dram_parameter("whh0t", [H, G4], bf16, isOutput=False)
    w0rep_d = nc.declare_dram_parameter("w0rep", [128, 128], bf16, isOutput=False)
    wih1t_d = nc.declare_dram_parameter("wih1t", [H, G4], bf16, isOutput=False)
    whh1t_d = nc.declare_dram_parameter("whh1t", [H, G4], bf16, isOutput=False)
    b0_d = nc.declare_dram_parameter("b0", [H, 4], f32, isOutput=False)
    b1_d = nc.declare_dram_parameter("b1", [H, 4], f32, isOutput=False)
    wht96_d = nc.declare_dram_parameter("wht96", [H, 96], bf16, isOutput=False)
    bh96_d = nc.declare_dram_parameter("bh96", [1, 96], bf16, isOutput=False)
    if scan_steps is None:
        scan_steps = t_steps
    n_hist = min(L_OUT, scan_steps)  # h1 steps kept for the head
    out_d = nc.declare_dram_parameter("out", [n_hist, 2, R], f32, isOutput=True)

    with tile.TileContext(nc) as tc:
        with (
            tc.tile_pool(name="consts", bufs=1) as consts,
            tc.tile_pool(name="xin", bufs=5) as xin,
            tc.tile_pool(name="gates", bufs=4, space="PSUM") as gpsum,
            tc.tile_pool(name="haccp", bufs=1) as haccp,
            tc.tile_pool(name="sig", bufs=12) as sigp,
            tc.tile_pool(name="tmp", bufs=6) as tmpp,
            tc.tile_pool(name="tcp", bufs=9) as tcp,
            tc.tile_pool(name="h0p", bufs=4) as h0p,
            tc.tile_pool(name="c0p", bufs=3) as c0p,
            tc.tile_pool(name="h1p", bufs=4) as h1p,
            tc.tile_pool(name="c1p", bufs=3) as c1p,
        ):
            # ---- load constants ----
            whh0t = consts.tile([H, G4], bf16)
            nc.sync.dma_start(out=whh0t[:], in_=whh0t_d[:])
            w0rep = consts.tile([128, 128], bf16)
            nc.sync.dma_start(out=w0rep[:], in_=w0rep_d[:])
            wih1t = consts.tile([H, G4], bf16)
            nc.sync.dma_start(out=wih1t[:], in_=wih1t_d[:])
            whh1t = consts.tile([H, G4], bf16)
            nc.sync.dma_start(out=whh1t[:], in_=whh1t_d[:])
            b0t = consts.tile([H, 4], f32)
            nc.sync.dma_start(out=b0t[:], in_=b0_d[:])
            b1t = consts.tile([H, 4], f32)
            nc.sync.dma_start(out=b1t[:], in_=b1_d[:])
            wht96 = consts.tile([H, 96], bf16)
            nc.sync.dma_start(out=wht96[:], in_=wht96_d[:])
            bh96 = consts.tile([1, 96], bf16)
            nc.sync.dma_start(out=bh96[:], in_=bh96_d[:])
            ones = consts.tile([1, R], bf16)
            nc.vector.memset(ones[:], 1.0)
            # per-partition intercept of the PWL center line for L1's o-gate:
            # u0 = 0.25*(x + b) + 0.5
            pwlb = consts.tile([H, 1], f32)
            nc.vector.tensor_scalar(
                pwlb[:], b1t[:, 3:4], 0.25, 0.5,
                mybir.AluOpType.mult, mybir.AluOpType.add,
            )

            def _scan_body():
                # ---- initial states ----
                h0 = h0p.tile([H, R], bf16)
                nc.vector.memset(h0[:], 0.0)
                c0 = c0p.tile([H, R], f32)
                nc.vector.memset(c0[:], 0.0)
                h1 = h1p.tile([H, R], bf16)
                nc.vector.memset(h1[:], 0.0)
                c1 = c1p.tile([H, R], f32)
                nc.vector.memset(c1[:], 0.0)

                CH = [(0, 512), (512, 128)]  # psum-bank-aligned column chunks

                # head accumulator in SBUF: mu at partitions 0..23,
                # sigma at 32..55. Each head step lands its (mu, sigma) rows
                # via a sliding 64-wide window into the zero-padded wht96/bh96
                # (transient PSUM tile), then folds into hacc on the DVE.
                hacc = haccp.tile([64, R], f32)

                def layer_head(gates_mm, bt, c_prev, cpool):
                    """MMs + sigmoids + p/q/c-update for one layer-step."""
                    gp = [
                        gpsum.tile([H, R], f32, tag="gates", name=f"gp{g}")
                        for g in range(4)
                    ]
                    for g in range(4):
                        gates_mm(gp[g], g)
                    sf = sigp.tile([H, R], f32, tag="sig")
                    nc.scalar.activation(sf[:], gp[0][:], AF.Sigmoid, bias=bt[:, 0:1])
                    q = tmpp.tile([H, R], f32, tag="tmp")
                    nc.gpsimd.tensor_mul(q[:], sf[:], c_prev[:])
                    tg = sigp.tile([H, R], f32, tag="sig")
                    nc.scalar.activation(tg[:], gp[1][:], AF.Tanh, bias=bt[:, 1:2])
                    si = sigp.tile([H, R], f32, tag="sig")
                    nc.scalar.activation(si[:], gp[2][:], AF.Sigmoid, bias=bt[:, 2:3])
                    # so/tc in bf16: they only feed h (already bf16) and unlock
                    # the DVE 2x mode for the h-multiply on the critical path
                    so = sigp.tile([H, R], bf16, tag="sigb")
                    nc.scalar.activation(so[:], gp[3][:], AF.Sigmoid, bias=bt[:, 3:4])
                    p = tmpp.tile([H, R], f32, tag="tmp")
                    c_new = cpool.tile([H, R], f32)
                    nc.vector.tensor_mul(p[:], si[:], tg[:])
                    nc.vector.tensor_add(c_new[:], p[:], q[:])
                    return c_new, so

                def layer_tail(c_new, so, hpool):
                    """tanh(c) and h = so*tanh(c)."""
                    tcv = tcp.tile([H, R], bf16, tag="tc")
                    h_new = hpool.tile([H, R], bf16)
                    nc.scalar.activation(tcv[:], c_new[:], AF.Tanh)
                    nc.vector.tensor_mul(h_new[:], so[:], tcv[:])
                    return h_new

                def head_mm(h1, s, first, last):
                    """Fold step s's (mu, sigma) rows into hacc via PSUM."""
                    hr = tmpp.tile([H, R], bf16, tag="hr")
                    nc.vector.tensor_scalar_max(hr[:], h1[:], 0.0)
                    hp = gpsum.tile([64, R], f32, tag="gates", name="hp")
                    base = 23 - s
                    for lo, w in CH:
                        nc.tensor.matmul(
                            hp[:, lo : lo + w],
                            lhsT=bh96[0:1, base : base + 64],
                            rhs=ones[0:1, lo : lo + w],
                            start=True,
                            stop=False,
                        )
                        nc.tensor.matmul(
                            hp[:, lo : lo + w],
                            lhsT=wht96[:, base : base + 64],
                            rhs=hr[:, lo : lo + w],
                            start=False,
                            stop=True,
                        )
                    if first:
                        nc.vector.tensor_copy(hacc[:], hp[:])
                    else:
                        nc.vector.tensor_add(hacc[:], hacc[:], hp[:])

                S = scan_steps

                def xt_dma(t):
                    xt = xin.tile([128, R], bf16, tag="x")
                    for g in range(4):
                        nc.sync.dma_start(
                            out=xt[32 * g : 32 * g + 5, :],
                            in_=xrep_d[t, 5 * g : 5 * g + 5, :],
                        )
                    return xt

                def l0_pass(gp, g, xt, h0):
                    for lo, w in CH:
                        nc.tensor.matmul(
                            gp[:, lo : lo + w],
                            lhsT=w0rep[32 * g : 32 * g + 5, :],
                            rhs=xt[32 * g : 32 * g + 5, lo : lo + w],
                            start=True,
                            stop=False,
                            tile_position=(32 * g, 0),
                        )
                    for lo, w in CH:
                        nc.tensor.matmul(
                            gp[:, lo : lo + w],
                            lhsT=whh0t[:, g * H : (g + 1) * H],
                            rhs=h0[:, lo : lo + w],
                            start=False,
                            stop=True,
                        )

                def l1_pass(gp, g, h0, h1):
                    # input part first (h0 is ready early), recurrent second
                    for lo, w in CH:
                        nc.tensor.matmul(
                            gp[:, lo : lo + w],
                            lhsT=wih1t[:, g * H : (g + 1) * H],
                            rhs=h0[:, lo : lo + w],
                            start=True,
                            stop=False,
                        )
                    for lo, w in CH:
                        nc.tensor.matmul(
                            gp[:, lo : lo + w],
                            lhsT=whh1t[:, g * H : (g + 1) * H],
                            rhs=h1[:, lo : lo + w],
                            start=False,
                            stop=True,
                        )

                # pipeline state
                h0s = {-1: h0}          # step -> h0 tile
                h1s = {-1: h1}
                c0s = {-1: c0}          # step -> c0 cell tile
                c1s = {-1: c1}
                gp0 = {}                # L0 gate psum tiles per pending step
                gp1 = {}
                sig0 = {}               # (sf, tg, si, so) per pending L0 step
                sig1 = {}
                xts = {}

                for t in range(min(2, S)):
                    xts[t] = xt_dma(t)

                # L1 lags L0 by two steps: iteration k runs the L1 sigmoids
                # for step k against L0 gates already two steps ahead, so no
                # ACT slot ever waits on the same-iteration h chain.
                for k in range(-2, S):
                    t1 = k          # L1 step activated this iter
                    tc = k + 1      # step whose c0 is tanh'd
                    t0 = k + 2      # L0 step whose gates are computed

                    if t0 + 1 < S and t0 + 1 not in xts:
                        xts[t0 + 1] = xt_dma(t0 + 1)

                    # ACT slot order: [F1, G1, C0, I1, O1, F0, C1, G0, I0, O0]
                    # sigma_f first so the Pool-engine q (slow on HW) has slack
                    # before the c-add needs it.
                    if 0 <= t1:
                        g1f, g1g, g1i, g1o = gp1.pop(t1)
                        sf1 = sigp.tile([H, R], f32, tag="sig")
                        nc.scalar.activation(sf1[:], g1f[:], AF.Sigmoid, bias=b1t[:, 0:1])
                        q1 = tmpp.tile([H, R], f32, tag="tmp")
                        nc.gpsimd.tensor_mul(q1[:], sf1[:], c1s[t1 - 1][:])
                        tg1 = sigp.tile([H, R], f32, tag="sig")
                        nc.scalar.activation(tg1[:], g1g[:], AF.Tanh, bias=b1t[:, 1:2])

                    # tanh(c0[tc]) and h0[tc]
                    if 0 <= tc < S:
                        tcv0 = tcp.tile([H, R], bf16, tag="tc")
                        nc.scalar.activation(tcv0[:], c0s[tc][:], AF.Tanh)
                        h0n = h0p.tile([H, R], bf16)
                        nc.vector.tensor_mul(h0n[:], sig0.pop(tc)[:], tcv0[:])
                        h0s[tc] = h0n

                    # sigma_i1, sigma_o1 + p1/c1 on DVE
                    if 0 <= t1:
                        si1 = sigp.tile([H, R], f32, tag="sig")
                        nc.scalar.activation(si1[:], g1i[:], AF.Sigmoid, bias=b1t[:, 2:3])
                        so1 = sigp.tile([H, R], bf16, tag="sigb")
                        nc.scalar.activation(so1[:], g1o[:], AF.Sigmoid, bias=b1t[:, 3:4])
                        p1 = tmpp.tile([H, R], f32, tag="tmp")
                        nc.vector.tensor_mul(p1[:], si1[:], tg1[:])
                        c1n = c1p.tile([H, R], f32)
                        nc.vector.tensor_add(c1n[:], p1[:], q1[:])
                        c1s[t1] = c1n

                    # sigma_f0 (L0 gates for t0 emitted here) + q0
                    if t0 < S:
                        gf = gpsum.tile([H, R], f32, tag="gates", name="g0f")
                        gg = gpsum.tile([H, R], f32, tag="gates", name="g0g")
                        gi = gpsum.tile([H, R], f32, tag="gates", name="g0i")
                        go = gpsum.tile([H, R], f32, tag="gates", name="g0o")
                        xt = xts.pop(t0)
                        for g, gp in enumerate((gf, gg, gi, go)):
                            l0_pass(gp, g, xt, h0s[t0 - 1])
                        sf0 = sigp.tile([H, R], f32, tag="sig")
                        nc.scalar.activation(sf0[:], gf[:], AF.Sigmoid, bias=b0t[:, 0:1])
                        q0 = tmpp.tile([H, R], f32, tag="tmp")
                        nc.gpsimd.tensor_mul(q0[:], sf0[:], c0s[t0 - 1][:])
                        gp0[t0] = (gg, gi, go)

                    # tanh(c1[t1]) + h1 + head + PE L1(t1+1)
                    if 0 <= t1:
                        tcv1 = tcp.tile([H, R], bf16, tag="tc")
                        nc.scalar.activation(tcv1[:], c1s[t1][:], AF.Tanh)
                        h1n = h1p.tile([H, R], bf16)
                        nc.vector.tensor_mul(h1n[:], so1[:], tcv1[:])
                        h1s[t1] = h1n
                        if t1 >= S - n_hist:
                            head_mm(h1n, t1 - (S - n_hist), t1 == S - n_hist, t1 == S - 1)
                    if 0 <= t1 + 1 < S:
                        p1f = gpsum.tile([H, R], f32, tag="gates", name="g1f")
                        p1g = gpsum.tile([H, R], f32, tag="gates", name="g1g")
                        p1i = gpsum.tile([H, R], f32, tag="gates", name="g1i")
                        p1o = gpsum.tile([H, R], f32, tag="gates", name="g1o")
                        for g, gp in enumerate((p1f, p1g, p1i, p1o)):
                            l1_pass(gp, g, h0s[t1 + 1], h1s[t1])
                        gp1[t1 + 1] = (p1f, p1g, p1i, p1o)

                    # tg0, sigma_i0, sigma_o0 + p0/c0 on DVE
                    if t0 < S:
                        gg, gi, go = gp0.pop(t0)
                        tg0 = sigp.tile([H, R], f32, tag="sig")
                        nc.scalar.activation(tg0[:], gg[:], AF.Tanh, bias=b0t[:, 1:2])
                        si0 = sigp.tile([H, R], f32, tag="sig")
                        nc.scalar.activation(si0[:], gi[:], AF.Sigmoid, bias=b0t[:, 2:3])
                        so0 = sigp.tile([H, R], bf16, tag="sigb")
                        nc.scalar.activation(so0[:], go[:], AF.Sigmoid, bias=b0t[:, 3:4])
                        sig0[t0] = so0
                        p0 = tmpp.tile([H, R], f32, tag="tmp")
                        nc.vector.tensor_mul(p0[:], si0[:], tg0[:])
                        c0n = c0p.tile([H, R], f32)
                        nc.vector.tensor_add(c0n[:], p0[:], q0[:])
                        c0s[t0] = c0n

                    # drop stale state refs so pools can recycle
                    for d in (h0s, h1s, c0s, c1s):
                        for key in [x for x in d if x < k - 2]:
                            d.pop(key)

                # ---- tail: softplus(sigma) in place, then output DMA ----
                sg = hacc[32 : 32 + n_hist, :]
                nc.scalar.activation(sg, sg, AF.Exp)
                nc.vector.tensor_scalar_add(sg, sg, 1.0)
                nc.scalar.activation(sg, sg, AF.Ln)
                nc.sync.dma_start(out=out_d[:, 0, :], in_=hacc[0:n_hist, :])
                nc.sync.dma_start(out=out_d[:, 1, :], in_=hacc[32 : 32 + n_hist, :])


            if repeat > 1:
                with tc.For_i(0, repeat, 1):
                    _scan_body()
            else:
                _scan_body()

    nc.compile()
    return nc


def _prepare_inputs(inputs: dict, t_steps: int):
    import ml_dtypes

    bf = ml_dtypes.bfloat16
    perm = _gate_perm()
    hist = np.asarray(inputs["history_data"], np.float32)
    fut = np.asarray(inputs["future_data"], np.float32)
    We = np.asarray(inputs["We"], np.float32)
    be = np.asarray(inputs["be"], np.float32)
    Wih0 = np.asarray(inputs["Wih0"], np.float32)
    Whh0 = np.asarray(inputs["Whh0"], np.float32)
    bih0 = np.asarray(inputs["bih0"], np.float32)
    bhh0 = np.asarray(inputs["bhh0"], np.float32)
    Wih1 = np.asarray(inputs["Wih1"], np.float32)
    Whh1 = np.asarray(inputs["Whh1"], np.float32)
    bih1 = np.asarray(inputs["bih1"], np.float32)
    bhh1 = np.asarray(inputs["bhh1"], np.float32)
    Wh = np.asarray(inputs["Wh"], np.float32)
    bh = np.asarray(inputs["bh"], np.float32)

    tgt = np.concatenate([hist[..., 0], fut[..., 0]], axis=1)      # [B, 120, N]
    cov = np.concatenate([hist[..., 1:], fut[..., 1:]], axis=1)    # [B, 120, N, COV]
    x5 = np.concatenate(
        [tgt[:, :t_steps, :, None], cov[:, 1 : t_steps + 1]], axis=-1
    )  # [B, T, N, 5]
    x5 = x5.transpose(1, 0, 2, 3).reshape(t_steps, BN, 5)

    W0eff = np.concatenate([Wih0[:, :E] @ We, Wih0[:, E:]], axis=1)  # [512, 5]
    b0 = bih0 + bhh0 + Wih0[:, :E] @ be
    b1 = bih1 + bhh1

    W0r = W0eff[perm]
    b0r = b0[perm]
    b1r = b1[perm]
    whh0t = np.ascontiguousarray(Whh0[perm].T).astype(bf)   # [128, 512]
    wih1t = np.ascontiguousarray(Wih1[perm].T).astype(bf)
    whh1t = np.ascontiguousarray(Whh1[perm].T).astype(bf)

    w0rep = np.zeros((128, 128), np.float32)
    w0t = W0r.T  # [5, 512]
    for g in range(4):
        w0rep[32 * g : 32 * g + 5, :] = w0t[:, g * H : (g + 1) * H]

    # head weights: wht96 is zero except col 23 = Wh[mu] and col 55 =
    # Wh[sigma]; head-step s reads the 64-wide window at col 23-s, landing
    # mu at output partition s and sigma at partition 32+s (32-aligned so
    # the tail Exp can address the sigma block)
    wht96 = np.zeros((H, 96), np.float32)
    wht96[:, 23] = Wh[0]
    wht96[:, 55] = Wh[1]
    bh96 = np.zeros((1, 96), np.float32)
    bh96[0, 23] = bh[0]
    bh96[0, 55] = bh[1]

    shared = {
        "whh0t": whh0t,
        "w0rep": w0rep.astype(bf),
        "wih1t": wih1t,
        "whh1t": whh1t,
        "b0": np.ascontiguousarray(b0r.reshape(4, H).T),
        "b1": np.ascontiguousarray(b1r.reshape(4, H).T),
        "wht96": wht96.astype(bf),
        "bh96": bh96.astype(bf),
    }
    in_maps = []
    for c in range(NCORES):
        xc = x5[:, c * R : (c + 1) * R, :]           # [T, R, 5]
        xt = np.ascontiguousarray(xc.transpose(0, 2, 1))  # [T, 5, R]
        xrep = np.tile(xt, (1, 4, 1))                # [T, 20, R]
        in_maps.append({"xrep": np.ascontiguousarray(xrep).astype(bf), **shared})
    return in_maps


def kernel(**inputs) -> np.ndarray:
    from concourse.bass_utils import run_bass_kernel_spmd

    t_steps = int(os.environ.get("DEEPAR_T_STEPS", T_STEPS))
    if t_steps not in _PROGRAM_CACHE:
        _PROGRAM_CACHE[t_steps] = _build_program(t_steps)
    nc = _PROGRAM_CACHE[t_steps]

    in_maps = _prepare_inputs(inputs, t_steps)
    res = run_bass_kernel_spmd(nc, in_maps, list(range(NCORES)))
    outs = [np.asarray(r["out"], np.float32) for r in res.results]
    full = np.concatenate(outs, axis=2)  # [n_hist, 2, BN]
    n_hist = full.shape[0]
    return np.ascontiguousarray(
        full.reshape(n_hist, 2, B, N_SER).transpose(2, 0, 3, 1)
    ).astype(np.float32)



# revision 21
# speedup vs baseline: 1.0763x; 1.0763x over previous
"""DeepAR 2-layer LSTM (B=16, L_IN=96, L_OUT=24, N=320, H=128) on 8 TRN2 cores.

Strategy (data-parallel over B*N=5120 rows, 640 rows/core):
  - Layout: hidden/gate index on SBUF partitions, batch rows on the free dim.
    h, c are [128, 640] tiles; gates are computed as W.T-slices (lhsT) against
    h (rhs) so no transposes are ever needed.
  - The input-side transform of layer 0 is folded through the embedding:
    x_eff[t] = [tgt_t, cov_{t+1}] (5 values/row) with W0eff = [Wih0[:, :E] @ We,
    Wih0[:, E:]] -> K=5 matmuls, packed 4-way with tile_position row strips.
  - Gate order is permuted to (f, g, i, o) host-side.
  - Software pipeline: layer 0 runs TWO steps ahead of layer 1, so every ACT
    (scalar-engine) instruction's inputs are produced at least one slot early;
    steady state runs the ACT engine gap-free at ~7.2us/step (ACT is the
    bottleneck: 10 sigmoid/tanh instructions x ~718ns).  Per-iteration ACT
    order: [sf1, tg1, tanh-c0, si1, so1, sf0, tanh-c1, tg0, si0, so0].
  - The mu/sigma head runs inline: relu(h1) feeds a PSUM-accumulated matmul
    whose zero-padded lhsT window lands step s at output partitions (s, 32+s),
    so no per-step copies/DMAs are needed; softplus(sigma) runs once at the
    end as exp/+1/ln.
  - Engines: PE matmuls (~60%); ACT sigmoid/tanh (~98% busy, the wall); DVE
    p=si*tg, c=p+q, h=so*tanh(c); GPSIMD computes q=sf*c_prev off the
    critical path (sigma_f is scheduled first to give it slack - the Q7 path
    is much slower on HW than the cost model suggests).
"""

import os

import numpy as np

B, L_IN, L_OUT, N_SER, COV = 16, 96, 24, 320, 4
E, H = 64, 128
T_STEPS = 119  # L_IN + L_OUT - 1
NCORES = 8
BN = B * N_SER          # 5120
R = BN // NCORES        # 640 rows per core
G4 = 4 * H              # 512 gates

_PROGRAM_CACHE: dict = {}


def _gate_perm() -> np.ndarray:
    # torch gate order in weights: i, f, g, o -> reorder rows to (f, g, i, o)
    # so the critical-path gates (forget, cell) come out of the PE first
    idx = np.arange(G4).reshape(4, H)
    return np.concatenate([idx[1], idx[2], idx[0], idx[3]])


def _build_program(t_steps: int, scan_steps: int | None = None, repeat: int = 1):
    import concourse.bacc as bacc
    import concourse.mybir as mybir
    import concourse.tile as tile

    f32 = mybir.dt.float32
    bf16 = mybir.dt.bfloat16
    AF = mybir.ActivationFunctionType

    nc = bacc.Bacc()

    xrep_d = nc.declare_dram_parameter("xrep", [t_steps, 20, R], bf16, isOutput=False)
    whh0t_d = nc.declare_dram_parameter("whh0t", [H, G4], bf16, isOutput=False)
    w0rep_d = nc.declare_dram_parameter("w0rep", [128, 128], bf16, isOutput=False)
    wih1t_d = nc.declare_dram_parameter("wih1t", [H, G4], bf16, isOutput=False)
    whh1t_d = nc.declare_dram_parameter("whh1t", [H, G4], bf16, isOutput=False)
    b0_d = nc.declare_dram_parameter("b0", [H, 4], f32, isOutput=False)
    b1_d = nc.declare_dram_parameter("b1", [H, 4], f32, isOutput=False)
    wht96_d = nc.declare_dram_parameter("wht96", [H, 96], bf16, isOutput=False)
    bh96_d = nc.declare_dram_parameter("bh96", [1, 96], bf16, isOutput=False)
    if scan_steps is None:
        scan_steps = t_steps
    n_hist = min(L_OUT, scan_steps)  # h1 steps kept for the head
    out_d = nc.declare_dram_parameter("out", [n_hist, 2, R], f32, isOutput=True)

    with tile.TileContext(nc) as tc:
        with (
            tc.tile_pool(name="consts", bufs=1) as consts,
            tc.tile_pool(name="xin", bufs=5) as xin,
            tc.tile_pool(name="gates", bufs=4, space="PSUM") as gpsum,
            tc.tile_pool(name="haccp", bufs=1) as haccp,
            tc.tile_pool(name="sig", bufs=12) as sigp,
            tc.tile_pool(name="tmp", bufs=6) as tmpp,
            tc.tile_pool(name="tcp", bufs=9) as tcp,
            tc.tile_pool(name="h0p", bufs=4) as h0p,
            tc.tile_pool(name="c0p", bufs=3) as c0p,
            tc.tile_pool(name="h1p", bufs=4) as h1p,
            tc.tile_pool(name="c1p", bufs=3) as c1p,
        ):
            # ---- load constants ----
            whh0t = consts.tile([H, G4], bf16)
            nc.sync.dma_start(out=whh0t[:], in_=whh0t_d[:])
            w0rep = consts.tile([128, 128], bf16)
            nc.sync.dma_start(out=w0rep[:], in_=w0rep_d[:])
            wih1t = consts.tile([H, G4], bf16)
            nc.sync.dma_start(out=wih1t[:], in_=wih1t_d[:])
            whh1t = consts.tile([H, G4], bf16)
            nc.sync.dma_start(out=whh1t[:], in_=whh1t_d[:])
            b0t = consts.tile([H, 4], f32)
            nc.sync.dma_start(out=b0t[:], in_=b0_d[:])
            b1t = consts.tile([H, 4], f32)
            nc.sync.dma_start(out=b1t[:], in_=b1_d[:])
            wht96 = consts.tile([H, 96], bf16)
            nc.sync.dma_start(out=wht96[:], in_=wht96_d[:])
            bh96 = consts.tile([1, 96], bf16)
            nc.sync.dma_start(out=bh96[:], in_=bh96_d[:])
            ones = consts.tile([1, R], bf16)
            nc.vector.memset(ones[:], 1.0)
            def _scan_body():
                # ---- initial states ----
                h0 = h0p.tile([H, R], bf16)
                nc.vector.memset(h0[:], 0.0)
                c0 = c0p.tile([H, R], f32)
                nc.vector.memset(c0[:], 0.0)
                h1 = h1p.tile([H, R], bf16)
                nc.vector.memset(h1[:], 0.0)
                c1 = c1p.tile([H, R], f32)
                nc.vector.memset(c1[:], 0.0)

                CH = [(0, 512), (512, 128)]  # psum-bank-aligned column chunks

                # head accumulator in SBUF: mu at partitions 0..23,
                # sigma at 32..55. Each head step lands its (mu, sigma) rows
                # via a sliding 64-wide window into the zero-padded wht96/bh96
                # (transient PSUM tile), then folds into hacc on the DVE.
                hacc = haccp.tile([64, R], f32)

                def head_mm(h1, s, first, last):
                    """Fold step s's (mu, sigma) rows into hacc via PSUM."""
                    hr = tmpp.tile([H, R], bf16, tag="hr")
                    nc.vector.tensor_scalar_max(hr[:], h1[:], 0.0)
                    hp = gpsum.tile([64, R], f32, tag="gates", name="hp")
                    base = 23 - s
                    for lo, w in CH:
                        nc.tensor.matmul(
                            hp[:, lo : lo + w],
                            lhsT=bh96[0:1, base : base + 64],
                            rhs=ones[0:1, lo : lo + w],
                            start=True,
                            stop=False,
                        )
                        nc.tensor.matmul(
                            hp[:, lo : lo + w],
                            lhsT=wht96[:, base : base + 64],
                            rhs=hr[:, lo : lo + w],
                            start=False,
                            stop=True,
                        )
                    if first:
                        nc.vector.tensor_copy(hacc[:], hp[:])
                    else:
                        nc.vector.tensor_add(hacc[:], hacc[:], hp[:])

                S = scan_steps

                def xt_dma(t):
                    xt = xin.tile([128, R], bf16, tag="x")
                    for g in range(4):
                        nc.sync.dma_start(
                            out=xt[32 * g : 32 * g + 5, :],
                            in_=xrep_d[t, 5 * g : 5 * g + 5, :],
                        )
                    return xt

                def l0_pass(gp, g, xt, h0):
                    for lo, w in CH:
                        nc.tensor.matmul(
                            gp[:, lo : lo + w],
                            lhsT=w0rep[32 * g : 32 * g + 5, :],
                            rhs=xt[32 * g : 32 * g + 5, lo : lo + w],
                            start=True,
                            stop=False,
                            tile_position=(32 * g, 0),
                        )
                    for lo, w in CH:
                        nc.tensor.matmul(
                            gp[:, lo : lo + w],
                            lhsT=whh0t[:, g * H : (g + 1) * H],
                            rhs=h0[:, lo : lo + w],
                            start=False,
                            stop=True,
                        )

                def l1_pass(gp, g, h0, h1):
                    # input part first (h0 is ready early), recurrent second
                    for lo, w in CH:
                        nc.tensor.matmul(
                            gp[:, lo : lo + w],
                            lhsT=wih1t[:, g * H : (g + 1) * H],
                            rhs=h0[:, lo : lo + w],
                            start=True,
                            stop=False,
                        )
                    for lo, w in CH:
                        nc.tensor.matmul(
                            gp[:, lo : lo + w],
                            lhsT=whh1t[:, g * H : (g + 1) * H],
                            rhs=h1[:, lo : lo + w],
                            start=False,
                            stop=True,
                        )

                # pipeline state
                h0s = {-1: h0}          # step -> h0 tile
                h1s = {-1: h1}
                c0s = {-1: c0}          # step -> c0 cell tile
                c1s = {-1: c1}
                gp0 = {}                # L0 gate psum tiles per pending step
                gp1 = {}
                sig0 = {}               # (sf, tg, si, so) per pending L0 step
                sig1 = {}
                xts = {}

                for t in range(min(2, S)):
                    xts[t] = xt_dma(t)

                # L1 lags L0 by two steps: iteration k runs the L1 sigmoids
                # for step k against L0 gates already two steps ahead, so no
                # ACT slot ever waits on the same-iteration h chain.
                for k in range(-2, S):
                    t1 = k          # L1 step activated this iter
                    tc = k + 1      # step whose c0 is tanh'd
                    t0 = k + 2      # L0 step whose gates are computed

                    if t0 + 1 < S and t0 + 1 not in xts:
                        xts[t0 + 1] = xt_dma(t0 + 1)

                    # ACT slot order: [F1, G1, C0, I1, O1, F0, C1, G0, I0, O0]
                    # sigma_f first so the Pool-engine q (slow on HW) has slack
                    # before the c-add needs it.
                    if 0 <= t1:
                        g1f, g1g, g1i, g1o = gp1.pop(t1)
                        sf1 = sigp.tile([H, R], f32, tag="sig")
                        nc.scalar.activation(sf1[:], g1f[:], AF.Sigmoid, bias=b1t[:, 0:1])
                        q1 = tmpp.tile([H, R], f32, tag="tmp")
                        nc.gpsimd.tensor_mul(q1[:], sf1[:], c1s[t1 - 1][:])
                        tg1 = sigp.tile([H, R], f32, tag="sig")
                        nc.scalar.activation(tg1[:], g1g[:], AF.Tanh, bias=b1t[:, 1:2])

                    # tanh(c0[tc]) and h0[tc]
                    if 0 <= tc < S:
                        tcv0 = tcp.tile([H, R], bf16, tag="tc")
                        nc.scalar.activation(tcv0[:], c0s[tc][:], AF.Tanh)
                        h0n = h0p.tile([H, R], bf16)
                        nc.vector.tensor_mul(h0n[:], sig0.pop(tc)[:], tcv0[:])
                        h0s[tc] = h0n

                    # sigma_i1, sigma_o1 + p1/c1 on DVE
                    if 0 <= t1:
                        si1 = sigp.tile([H, R], f32, tag="sig")
                        nc.scalar.activation(si1[:], g1i[:], AF.Sigmoid, bias=b1t[:, 2:3])
                        so1 = sigp.tile([H, R], bf16, tag="sigb")
                        nc.scalar.activation(so1[:], g1o[:], AF.Sigmoid, bias=b1t[:, 3:4])
                        p1 = tmpp.tile([H, R], f32, tag="tmp")
                        nc.vector.tensor_mul(p1[:], si1[:], tg1[:])
                        c1n = c1p.tile([H, R], f32)
                        nc.vector.tensor_add(c1n[:], p1[:], q1[:])
                        c1s[t1] = c1n

                    # sigma_f0 (L0 gates for t0 emitted here) + q0
                    if t0 < S:
                        gf = gpsum.tile([H, R], f32, tag="gates", name="g0f")
                        gg = gpsum.tile([H, R], f32, tag="gates", name="g0g")
                        gi = gpsum.tile([H, R], f32, tag="gates", name="g0i")
                        go = gpsum.tile([H, R], f32, tag="gates", name="g0o")
                        xt = xts.pop(t0)
                        for g, gp in enumerate((gf, gg, gi, go)):
                            l0_pass(gp, g, xt, h0s[t0 - 1])
                        sf0 = sigp.tile([H, R], f32, tag="sig")
                        nc.scalar.activation(sf0[:], gf[:], AF.Sigmoid, bias=b0t[:, 0:1])
                        q0 = tmpp.tile([H, R], f32, tag="tmp")
                        nc.gpsimd.tensor_mul(q0[:], sf0[:], c0s[t0 - 1][:])
                        gp0[t0] = (gg, gi, go)

                    # tanh(c1[t1]) + h1 + head + PE L1(t1+1)
                    if 0 <= t1:
                        tcv1 = tcp.tile([H, R], bf16, tag="tc")
                        nc.scalar.activation(tcv1[:], c1s[t1][:], AF.Tanh)
                        h1n = h1p.tile([H, R], bf16)
                        nc.vector.tensor_mul(h1n[:], so1[:], tcv1[:])
                        h1s[t1] = h1n
                        if t1 >= S - n_hist:
                            head_mm(h1n, t1 - (S - n_hist), t1 == S - n_hist, t1 == S - 1)
                    if 0 <= t1 + 1 < S:
                        p1f = gpsum.tile([H, R], f32, tag="gates", name="g1f")
                        p1g = gpsum.tile([H, R], f32, tag="gates", name="g1g")
                        p1i = gpsum.tile([H, R], f32, tag="gates", name="g1i")
                        p1o = gpsum.tile([H, R], f32, tag="gates", name="g1o")
                        for g, gp in enumerate((p1f, p1g, p1i, p1o)):
                            l1_pass(gp, g, h0s[t1 + 1], h1s[t1])
                        gp1[t1 + 1] = (p1f, p1g, p1i, p1o)

                    # tg0, sigma_i0, sigma_o0 + p0/c0 on DVE
                    if t0 < S:
                        gg, gi, go = gp0.pop(t0)
                        tg0 = sigp.tile([H, R], f32, tag="sig")
                        nc.scalar.activation(tg0[:], gg[:], AF.Tanh, bias=b0t[:, 1:2])
                        si0 = sigp.tile([H, R], f32, tag="sig")
                        nc.scalar.activation(si0[:], gi[:], AF.Sigmoid, bias=b0t[:, 2:3])
                        so0 = sigp.tile([H, R], bf16, tag="sigb")
                        nc.scalar.activation(so0[:], go[:], AF.Sigmoid, bias=b0t[:, 3:4])
                        sig0[t0] = so0
                        p0 = tmpp.tile([H, R], f32, tag="tmp")
                        nc.vector.tensor_mul(p0[:], si0[:], tg0[:])
                        c0n = c0p.tile([H, R], f32)
                        nc.vector.tensor_add(c0n[:], p0[:], q0[:])
                        c0s[t0] = c0n

                    # drop stale state refs so pools can recycle
                    for d in (h0s, h1s, c0s, c1s):
                        for key in [x for x in d if x < k - 2]:
                            d.pop(key)

                # ---- tail: softplus(sigma) in place, then output DMA ----
                sg = hacc[32 : 32 + n_hist, :]
                nc.scalar.activation(sg, sg, AF.Exp)
                nc.vector.tensor_scalar_add(sg, sg, 1.0)
                nc.scalar.activation(sg, sg, AF.Ln)
                nc.sync.dma_start(out=out_d[:, 0, :], in_=hacc[0:n_hist, :])
                nc.sync.dma_start(out=out_d[:, 1, :], in_=hacc[32 : 32 + n_hist, :])


            if repeat > 1:
                with tc.For_i(0, repeat, 1):
                    _scan_body()
            else:
                _scan_body()

    nc.compile()
    return nc


def _prepare_inputs(inputs: dict, t_steps: int):
    import ml_dtypes

    bf = ml_dtypes.bfloat16
    perm = _gate_perm()
    hist = np.asarray(inputs["history_data"], np.float32)
    fut = np.asarray(inputs["future_data"], np.float32)
    We = np.asarray(inputs["We"], np.float32)
    be = np.asarray(inputs["be"], np.float32)
    Wih0 = np.asarray(inputs["Wih0"], np.float32)
    Whh0 = np.asarray(inputs["Whh0"], np.float32)
    bih0 = np.asarray(inputs["bih0"], np.float32)
    bhh0 = np.asarray(inputs["bhh0"], np.float32)
    Wih1 = np.asarray(inputs["Wih1"], np.float32)
    Whh1 = np.asarray(inputs["Whh1"], np.float32)
    bih1 = np.asarray(inputs["bih1"], np.float32)
    bhh1 = np.asarray(inputs["bhh1"], np.float32)
    Wh = np.asarray(inputs["Wh"], np.float32)
    bh = np.asarray(inputs["bh"], np.float32)

    tgt = np.concatenate([hist[..., 0], fut[..., 0]], axis=1)      # [B, 120, N]
    cov = np.concatenate([hist[..., 1:], fut[..., 1:]], axis=1)    # [B, 120, N, COV]
    x5 = np.concatenate(
        [tgt[:, :t_steps, :, None], cov[:, 1 : t_steps + 1]], axis=-1
    )  # [B, T, N, 5]
    x5 = x5.transpose(1, 0, 2, 3).reshape(t_steps, BN, 5)

    W0eff = np.concatenate([Wih0[:, :E] @ We, Wih0[:, E:]], axis=1)  # [512, 5]
    b0 = bih0 + bhh0 + Wih0[:, :E] @ be
    b1 = bih1 + bhh1

    W0r = W0eff[perm]
    b0r = b0[perm]
    b1r = b1[perm]
    whh0t = np.ascontiguousarray(Whh0[perm].T).astype(bf)   # [128, 512]
    wih1t = np.ascontiguousarray(Wih1[perm].T).astype(bf)
    whh1t = np.ascontiguousarray(Whh1[perm].T).astype(bf)

    w0rep = np.zeros((128, 128), np.float32)
    w0t = W0r.T  # [5, 512]
    for g in range(4):
        w0rep[32 * g : 32 * g + 5, :] = w0t[:, g * H : (g + 1) * H]

    # head weights: wht96 is zero except col 23 = Wh[mu] and col 55 =
    # Wh[sigma]; head-step s reads the 64-wide window at col 23-s, landing
    # mu at output partition s and sigma at partition 32+s (32-aligned so
    # the tail Exp can address the sigma block)
    wht96 = np.zeros((H, 96), np.float32)
    wht96[:, 23] = Wh[0]
    wht96[:, 55] = Wh[1]
    bh96 = np.zeros((1, 96), np.float32)
    bh96[0, 23] = bh[0]
    bh96[0, 55] = bh[1]

    shared = {
        "whh0t": whh0t,
        "w0rep": w0rep.astype(bf),
        "wih1t": wih1t,
        "whh1t": whh1t,
        "b0": np.ascontiguousarray(b0r.reshape(4, H).T),
        "b1": np.ascontiguousarray(b1r.reshape(4, H).T),
        "wht96": wht96.astype(bf),
        "bh96": bh96.astype(bf),
    }
    in_maps = []
    for c in range(NCORES):
        xc = x5[:, c * R : (c + 1) * R, :]           # [T, R, 5]
        xt = np.ascontiguousarray(xc.transpose(0, 2, 1))  # [T, 5, R]
        xrep = np.tile(xt, (1, 4, 1))                # [T, 20, R]
        in_maps.append({"xrep": np.ascontiguousarray(xrep).astype(bf), **shared})
    return in_maps


def kernel(**inputs) -> np.ndarray:
    from concourse.bass_utils import run_bass_kernel_spmd

    t_steps = int(os.environ.get("DEEPAR_T_STEPS", T_STEPS))
    if t_steps not in _PROGRAM_CACHE:
        _PROGRAM_CACHE[t_steps] = _build_program(t_steps)
    nc = _PROGRAM_CACHE[t_steps]

    in_maps = _prepare_inputs(inputs, t_steps)
    res = run_bass_kernel_spmd(nc, in_maps, list(range(NCORES)))
    outs = [np.asarray(r["out"], np.float32) for r in res.results]
    full = np.concatenate(outs, axis=2)  # [n_hist, 2, BN]
    n_hist = full.shape[0]
    return np.ascontiguousarray(
        full.reshape(n_hist, 2, B, N_SER).transpose(2, 0, 3, 1)
    ).astype(np.float32)

